# revision 9
# baseline (speedup 1.0000x reference)
"""GPT forward (L=8, D=1024, H=16, T=1024, B=2, V=50257) on 8 TRN2 NeuronCores.

Sharding: token-sharded layers (sequence parallel) — core c handles batch c//4,
token slice (c%4)*256..+256, weights replicated (bf16), one 4-core grouped
AllGather of (k, v^T) per layer for attention, vocab-sharded fp32r LM head
after an AllGather of the final layernormed activations. Residual stream fp32r,
matmul accumulation fp32 in PSUM.

kernel(**inputs) takes the FULL unsharded inputs (as from setup_inputs()) and
returns (logits [B,T,V] f32, loss scalar f32) like the reference.
"""

import sys

if "/opt/trn_rl_repo" not in sys.path:
    sys.path.insert(0, "/opt/trn_rl_repo")

import numpy as np
import ml_dtypes

import concourse.bass as bass
from concourse import bacc
import concourse.mybir as mybir
import concourse.tile as tile
from concourse.bass_utils import run_bass_kernel_spmd

V, D, H, L, T, B = 50257, 1024, 16, 8, 1024, 2
DH = D // H
DF = 4 * D
P = 128
TS = T // 4  # 256 tokens per core
KT = D // P  # 8
GROUPS = [[0, 1, 2, 3], [4, 5, 6, 7]]

VS0 = 12565
V_OFFS = [0, VS0, VS0 + 12564, VS0 + 2 * 12564]
V_LENS = [12565, 12564, 12564, 12564]
VPAD = 12800
VCH = VPAD // 512  # 25

F32 = mybir.dt.float32
F32R = mybir.dt.float32r
BF16 = mybir.dt.bfloat16

MGROUP = 4

_cache = {}


def _build_nc():
    if "nc" in _cache:
        return _cache["nc"]
    nc = bacc.Bacc()

    dp = nc.declare_dram_parameter
    x0_in = dp("x0", [D, TS], F32, isOutput=False)
    wqkv_in = dp("wqkv", [L, D, 3 * D], BF16, isOutput=False)
    wout_in = dp("wout", [L, D, D], BF16, isOutput=False)
    wff1_in = dp("wff1", [L, D, DF], BF16, isOutput=False)
    wff2_in = dp("wff2", [L, DF, D], BF16, isOutput=False)
    bqkv_in = dp("bqkv", [P, L * 24], F32, isOutput=False)
    bout_in = dp("bout", [P, L * KT], F32, isOutput=False)
    bff1_in = dp("bff1", [P, L * 32], F32, isOutput=False)
    bff2_in = dp("bff2", [P, L * KT], F32, isOutput=False)
    ln1w_in = dp("ln1w", [P, L * KT], F32, isOutput=False)
    ln1b_in = dp("ln1b", [P, L * KT], F32, isOutput=False)
    ln2w_in = dp("ln2w", [P, L * KT], F32, isOutput=False)
    ln2b_in = dp("ln2b", [P, L * KT], F32, isOutput=False)
    lnfw_in = dp("lnfw", [P, KT], F32, isOutput=False)
    lnfb_in = dp("lnfb", [P, KT], F32, isOutput=False)
    maskt_in = dp("maskt", [P, 4, 2, TS], BF16, isOutput=False)
    embt_in = dp("embt", [D, VPAD], F32, isOutput=False)
    blm_in = dp("blm", [P, VPAD], BF16, isOutput=False)  # pre-broadcast on host
    ones_in = dp("ones1", [P, 1], F32, isOutput=False)
    identb_in = dp("identb", [P, P], BF16, isOutput=False)

    logits_out = dp("logits", [T, VPAD], F32, isOutput=True)
    sumexp_out = dp("sumexp", [8, P], F32, isOutput=True)

    kv_cc_in = [nc.dram_tensor(f"kvin{l}", [2 * D, TS], BF16) for l in range(L)]
    kv_cc_out = [nc.dram_tensor(f"kvout{l}", [8 * D, TS], BF16) for l in range(L)]
    xl_cc_in = nc.dram_tensor("xlin", [D, TS], F32)
    xl_cc_out = nc.dram_tensor("xlout", [4 * D, TS], F32)

    eps = 1e-5

    with tile.TileContext(nc) as tc:
        with (
            tc.tile_pool(name="const", bufs=1) as constp,
            tc.tile_pool(name="acts", bufs=1) as acts,
            tc.tile_pool(name="small", bufs=2) as small,
            tc.tile_pool(name="drains", bufs=4) as drains,
            tc.tile_pool(name="ps_mm", bufs=MGROUP, space="PSUM") as ps_mm,
            tc.tile_pool(name="ps_sc", bufs=2, space="PSUM") as ps_sc,
            tc.tile_pool(name="ps_o", bufs=1, space="PSUM") as ps_o,
            tc.tile_pool(name="ps_sm", bufs=1, space="PSUM") as ps_sm,
        ):
            # ---- constants ----
            ones_r = constp.tile([P, 1], F32R)
            nc.sync.dma_start(ones_r[:], ones_in[:].bitcast(F32R))
            identb = constp.tile([P, P], BF16)
            nc.sync.dma_start(identb[:], identb_in[:])
            ones_b = constp.tile([P, 1], BF16)
            nc.vector.memset(ones_b[:], 1.0)
            maskt = constp.tile([P, 4, 2, TS], BF16)
            nc.sync.dma_start(maskt[:], maskt_in[:])
            params = {}
            for name, t_in, width in (
                ("ln1w", ln1w_in, KT),
                ("ln1b", ln1b_in, KT),
                ("ln2w", ln2w_in, KT),
                ("ln2b", ln2b_in, KT),
                ("bqkv", bqkv_in, 24),
                ("bout", bout_in, KT),
                ("bff1", bff1_in, 32),
                ("bff2", bff2_in, KT),
            ):
                tl = constp.tile([P, L * width], F32, tag=name, name=name + "_t")
                nc.sync.dma_start(tl[:], t_in[:])
                params[name] = (tl, width)
            lnfw_t = constp.tile([P, KT], F32)
            nc.sync.dma_start(lnfw_t[:], lnfw_in[:])
            lnfb_t = constp.tile([P, KT], F32)
            nc.sync.dma_start(lnfb_t[:], lnfb_in[:])

            def pslice(name, l):
                tl, width = params[name]
                return tl[:, l * width : (l + 1) * width]

            # ---- residual ----
            x = acts.tile([P, KT, TS], F32R)
            nc.sync.dma_start(
                x[:], x0_in[:].rearrange("(ko p) t -> p ko t", p=P).bitcast(F32R)
            )

            def make_layernorm(wrk):
                def layernorm_into(h_out, wvec, bvec):
                    xsq = wrk.tile([P, KT, TS], F32R, tag="xsq")
                    nc.scalar.activation(
                        xsq[:], x[:].bitcast(F32), mybir.ActivationFunctionType.Square
                    )
                    s1 = ps_sc.tile([1, TS], F32, tag="sc")
                    for k in range(KT):
                        nc.tensor.matmul(
                            s1[:], ones_r[:], x[:, k, :],
                            start=(k == 0), stop=(k == KT - 1),
                        )
                    s2 = ps_sc.tile([1, TS], F32, tag="sc")
                    for k in range(KT):
                        nc.tensor.matmul(
                            s2[:], ones_r[:], xsq[:, k, :],
                            start=(k == 0), stop=(k == KT - 1),
                        )
                    stat = small.tile([1, 4, TS], F32, tag="stat")
                    mean = stat[:, 0, :]
                    msq = stat[:, 1, :]
                    a_ = stat[:, 2, :]
                    b_ = stat[:, 3, :]
                    nc.vector.tensor_scalar_mul(mean, s1[:], 1.0 / D)
                    nc.vector.tensor_scalar_mul(msq, s2[:], 1.0 / D)
                    nc.vector.tensor_tensor(b_, mean, mean, op=mybir.AluOpType.mult)
                    nc.vector.tensor_tensor(a_, msq, b_, op=mybir.AluOpType.subtract)
                    nc.vector.tensor_scalar_add(a_, a_, eps)
                    nc.scalar.activation(a_, a_, mybir.ActivationFunctionType.Sqrt)
                    nc.vector.reciprocal(a_, a_)
                    nc.vector.tensor_tensor(b_, mean, a_, op=mybir.AluOpType.mult)
                    nc.vector.tensor_scalar_mul(b_, b_, -1.0)
                    ab_bc = small.tile([P, 2, TS], F32, tag="abbc")
                    nc.gpsimd.partition_broadcast(ab_bc[:, 0, :], a_)
                    nc.gpsimd.partition_broadcast(ab_bc[:, 1, :], b_)
                    tmp = wrk.tile([P, KT, TS], F32, tag="lntmp")
                    for k in range(KT):
                        nc.vector.tensor_tensor(
                            tmp[:, k, :], x[:, k, :].bitcast(F32), ab_bc[:, 0, :],
                            op=mybir.AluOpType.mult,
                        )
                        nc.vector.tensor_tensor(
                            tmp[:, k, :], tmp[:, k, :], ab_bc[:, 1, :],
                            op=mybir.AluOpType.add,
                        )
                        nc.scalar.activation(
                            h_out[:, k, :], tmp[:, k, :],
                            mybir.ActivationFunctionType.Identity,
                            bias=bvec[:, k : k + 1], scale=wvec[:, k : k + 1],
                        )

                return layernorm_into

            def make_matmul_block(wstream):
                def matmul_block(out_cb, w_dram, KTILES, MT, rhs, l):
                    n_mg = (MT + MGROUP - 1) // MGROUP
                    for mg in range(n_mg):
                        m0 = mg * MGROUP
                        m1 = min(m0 + MGROUP, MT)
                        nm = m1 - m0
                        pts = [ps_mm.tile([P, TS], F32, tag="mm", name=f"mmps{mg}_{i}") for i in range(nm)]
                        for k in range(KTILES):
                            wt = wstream.tile([P, MGROUP * P], BF16, tag="w")
                            nc.sync.dma_start(
                                wt[:, : nm * P],
                                w_dram[l, k * P : (k + 1) * P, m0 * P : m1 * P],
                            )
                            for mi in range(nm):
                                nc.tensor.matmul(
                                    pts[mi][:],
                                    wt[:, mi * P : (mi + 1) * P],
                                    rhs[:, k, :],
                                    start=(k == 0),
                                    stop=(k == KTILES - 1),
                                )
                        for mi in range(nm):
                            out_cb(m0 + mi, pts[mi])

                return matmul_block

            # ================= layers (own pool scope) =================
            with (
                tc.tile_pool(name="wrk", bufs=1) as wrk,
                tc.tile_pool(name="wstream", bufs=3) as wstream,
                tc.tile_pool(name="probs", bufs=2) as probsp,
            ):
                layernorm_into = make_layernorm(wrk)
                matmul_block = make_matmul_block(wstream)

                for l in range(L):
                    # ---------------- LN1 -> h ----------------
                    h = wrk.tile([P, KT, TS], BF16, tag="h")
                    layernorm_into(h, pslice("ln1w", l), pslice("ln1b", l))

                    # ---------------- qkv ----------------
                    qkv = wrk.tile([P, 24, TS], BF16, tag="qkv")
                    bq = params["bqkv"][0]

                    def qkv_drain(m, pt, l=l, qkv=qkv, bq=bq):
                        nc.scalar.activation(
                            qkv[:, m, :], pt[:],
                            mybir.ActivationFunctionType.Identity,
                            bias=bq[:, l * 24 + m : l * 24 + m + 1],
                        )

                    matmul_block(qkv_drain, wqkv_in, KT, 24, h, l)

                    # ------------- pack (k, v^T), AllGather -------------
                    nc.sync.dma_start(
                        kv_cc_in[l][0:D, :].rearrange("(o p) t -> p o t", p=P),
                        qkv[:, 8:16, :],
                    )
                    vt_mine = wrk.tile([P, 2, D], BF16, tag="vtm")
                    for dt in range(KT):
                        for tt in range(2):
                            ptp = ps_sm.tile([P, P], BF16, tag="small")
                            nc.tensor.transpose(
                                ptp[:], qkv[:, 16 + dt, tt * P : (tt + 1) * P],
                                identb[:],
                            )
                            nc.vector.tensor_copy(
                                out=vt_mine[:, tt, dt * P : (dt + 1) * P], in_=ptp[:]
                            )
                    nc.sync.dma_start(
                        kv_cc_in[l][D : 2 * D, :]
                        .rearrange("(a b) c -> a (b c)", a=TS)
                        .rearrange("(tt p) d -> p tt d", p=P),
                        vt_mine[:],
                    )
                    nc.gpsimd.collective_compute(
                        "AllGather",
                        mybir.AluOpType.bypass,
                        replica_groups=GROUPS,
                        ins=[kv_cc_in[l][:]],
                        outs=[kv_cc_out[l][:]],
                    )
                    kg = wrk.tile([P, KT, 4, TS], BF16, tag="kg")
                    vt_pad = wrk.tile([P, 2, 4, H, 64], BF16, tag="vtp")
                    for r in range(4):
                        base = r * 2 * D
                        nc.sync.dma_start(
                            kg[:, :, r, :],
                            kv_cc_out[l][base : base + D, :].rearrange(
                                "(o p) t -> p o t", p=P
                            ),
                        )
                        nc.sync.dma_start(
                            vt_pad[:, :, r, :, :],
                            kv_cc_out[l][base + D : base + 2 * D, :]
                            .rearrange("(a b) c -> a (b c)", a=TS)
                            .rearrange("(tt p) (hh d) -> p tt hh d", p=P, d=64),
                        )

                    # ---------------- attention ----------------
                    o = wrk.tile([P, KT, TS], BF16, tag="o")
                    for hh in range(H):
                        ph = (hh % 2) * 64
                        dt = hh // 2
                        probs = probsp.tile([P, 8, TS], BF16, tag="pr")
                        for r in range(4):
                            for tt in range(2):
                                rj = r * 2 + tt
                                psc = ps_sc.tile([P, TS], F32, tag="sc")
                                nc.tensor.matmul(
                                    psc[:],
                                    kg[ph : ph + 64, dt, r, tt * P : (tt + 1) * P],
                                    qkv[ph : ph + 64, dt, :],
                                    start=True,
                                    stop=True,
                                )
                                nc.scalar.activation(
                                    probs[:, rj, :], psc[:],
                                    mybir.ActivationFunctionType.Exp,
                                )
                                nc.vector.tensor_tensor(
                                    probs[:, rj, :], probs[:, rj, :],
                                    maskt[:, r, tt, :],
                                    op=mybir.AluOpType.mult,
                                )
                        po = ps_o.tile([64, TS], F32, tag="o")
                        pse = ps_sm.tile([1, TS], F32, tag="small")
                        for r in range(4):
                            for tt in range(2):
                                rj = r * 2 + tt
                                nc.tensor.matmul(
                                    po[:],
                                    vt_pad[:, tt, r, hh, :],
                                    probs[:, rj, :],
                                    start=(rj == 0),
                                    stop=(rj == 7),
                                )
                                nc.tensor.matmul(
                                    pse[:],
                                    ones_b[:],
                                    probs[:, rj, :],
                                    start=(rj == 0),
                                    stop=(rj == 7),
                                )
                        rec = small.tile([1, TS], F32, tag="rec")
                        nc.vector.reciprocal(rec[:], pse[:])
                        rec_bc = small.tile([64, TS], F32, tag="recbc")
                        nc.gpsimd.partition_broadcast(rec_bc[:], rec[:])
                        o_tmp = drains.tile([64, TS], BF16, tag="otmp")
                        nc.vector.tensor_tensor(
                            o_tmp[:], po[:], rec_bc[:],
                            op=mybir.AluOpType.mult,
                        )
                        nc.sync.dma_start(o[ph : ph + 64, dt, :], o_tmp[:])

                    # ---------------- out proj + residual ----------------
                    def out_drain(m, pt, l=l):
                        tmp = drains.tile([P, TS], F32, tag="ytmp")
                        nc.scalar.activation(
                            tmp[:], pt[:], mybir.ActivationFunctionType.Identity,
                            bias=pslice("bout", l)[:, m : m + 1],
                        )
                        nc.vector.tensor_tensor(
                            x[:, m, :], x[:, m, :].bitcast(F32), tmp[:],
                            op=mybir.AluOpType.add,
                        )

                    matmul_block(out_drain, wout_in, KT, KT, o, l)

                    # ---------------- LN2 -> h2, ff1 ----------------
                    h2 = wrk.tile([P, KT, TS], BF16, tag="h")
                    layernorm_into(h2, pslice("ln2w", l), pslice("ln2b", l))
                    ff = wrk.tile([P, 32, TS], BF16, tag="qkv")

                    def ff1_drain(m, pt, l=l, ff=ff):
                        nc.scalar.activation(
                            ff[:, m, :], pt[:], mybir.ActivationFunctionType.Gelu,
                            bias=pslice("bff1", l)[:, m : m + 1],
                        )

                    matmul_block(ff1_drain, wff1_in, KT, 32, h2, l)

                    # ---------------- ff2 + residual ----------------
                    def ff2_drain(m, pt, l=l):
                        tmp = drains.tile([P, TS], F32, tag="ytmp")
                        nc.scalar.activation(
                            tmp[:], pt[:], mybir.ActivationFunctionType.Identity,
                            bias=pslice("bff2", l)[:, m : m + 1],
                        )
                        nc.vector.tensor_tensor(
                            x[:, m, :], x[:, m, :].bitcast(F32), tmp[:],
                            op=mybir.AluOpType.add,
                        )

                    matmul_block(ff2_drain, wff2_in, 32, KT, ff, l)

                # ---- final LN + AllGather x_lnf ----
                xl = wrk.tile([P, KT, TS], F32R, tag="h")
                layernorm_into(xl, lnfw_t[:], lnfb_t[:])
                nc.sync.dma_start(
                    xl_cc_in[:].rearrange("(o p) t -> p o t", p=P),
                    xl[:].bitcast(F32),
                )
                nc.gpsimd.collective_compute(
                    "AllGather",
                    mybir.AluOpType.bypass,
                    replica_groups=GROUPS,
                    ins=[xl_cc_in[:]],
                    outs=[xl_cc_out[:]],
                )

            # ================= LM head (own pool scope) =================
            with tc.tile_pool(name="lmp", bufs=1) as lmp, tc.tile_pool(
                name="embs", bufs=2
            ) as embs:
                xlg = lmp.tile([P, 4, KT, TS], F32R)
                nc.sync.dma_start(
                    xlg[:],
                    xl_cc_out[:]
                    .rearrange("(r o p) t -> p r o t", p=P, o=KT)
                    .bitcast(F32R),
                )
                bias_bc = lmp.tile([P, VPAD], BF16)
                nc.sync.dma_start(bias_bc[:], blm_in[:])
                sump = lmp.tile([P, 8, VCH], F32)
                for vc in range(VCH):
                    et = embs.tile([P, KT, 512], F32R, tag="emb")
                    nc.sync.dma_start(
                        et[:],
                        embt_in[:, vc * 512 : (vc + 1) * 512]
                        .rearrange("(o p) c -> p o c", p=P)
                        .bitcast(F32R),
                    )
                    for r in range(4):
                        for tt in range(2):
                            ti = r * 2 + tt
                            pl = ps_mm.tile([P, 512], F32, tag="mm")
                            for k in range(KT):
                                nc.tensor.matmul(
                                    pl[:],
                                    xlg[:, r, k, tt * P : (tt + 1) * P],
                                    et[:, k, :],
                                    start=(k == 0),
                                    stop=(k == KT - 1),
                                )
                            lsb = drains.tile([P, 512], F32, tag="lg")
                            nc.vector.tensor_tensor(
                                lsb[:], pl[:],
                                bias_bc[:, vc * 512 : (vc + 1) * 512],
                                op=mybir.AluOpType.add,
                            )
                            nc.sync.dma_start(
                                logits_out[
                                    ti * P : (ti + 1) * P, vc * 512 : (vc + 1) * 512
                                ],
                                lsb[:],
                            )
                            esc = drains.tile([P, 512], F32, tag="esc")
                            nc.scalar.activation(
                                esc[:], lsb[:], mybir.ActivationFunctionType.Exp,
                                accum_out=sump[:, ti, vc : vc + 1],
                            )
                stile = lmp.tile([P, 8], F32)
                for ti in range(8):
                    nc.vector.tensor_reduce(
                        stile[:, ti : ti + 1], sump[:, ti, :],
                        mybir.AxisListType.X, mybir.AluOpType.add,
                    )
                nc.sync.dma_start(sumexp_out[:].rearrange("a p -> p a"), stile[:])

    nc.finalize()
    _cache["nc"] = nc
    return nc


def _prep_host(inputs):
    f32 = np.float32
    bf16 = ml_dtypes.bfloat16
    idx = np.asarray(inputs["idx"])
    targets = np.asarray(inputs["targets"])
    tok_emb = np.asarray(inputs["tok_emb"], dtype=f32)
    pos_emb = np.asarray(inputs["pos_emb"], dtype=f32)

    qkv_wT = np.ascontiguousarray(
        np.transpose(np.asarray(inputs["qkv_w"], dtype=f32), (0, 2, 1))
    )
    qkv_wT[:, :, :D] *= f32(1.0 / np.sqrt(DH))
    bqkv = np.array(np.asarray(inputs["qkv_b"], dtype=f32))
    bqkv[:, :D] *= f32(1.0 / np.sqrt(DH))
    def featmaj(a, width):
        # [L, M] -> [P, L*width] where column l*width+mo holds a[l, mo*128+p]
        a = np.asarray(a, dtype=f32)
        Lw = a.shape[0]
        out = a.reshape(Lw, width, P).transpose(2, 0, 1).reshape(P, Lw * width)
        return np.ascontiguousarray(out)

    shared = {
        "wqkv": qkv_wT.astype(bf16),
        "wout": np.ascontiguousarray(
            np.transpose(np.asarray(inputs["out_w"], dtype=f32), (0, 2, 1))
        ).astype(bf16),
        "wff1": np.ascontiguousarray(
            np.transpose(np.asarray(inputs["ff1_w"], dtype=f32), (0, 2, 1))
        ).astype(bf16),
        "wff2": np.ascontiguousarray(
            np.transpose(np.asarray(inputs["ff2_w"], dtype=f32), (0, 2, 1))
        ).astype(bf16),
        "bqkv": featmaj(bqkv, 24),
        "bout": featmaj(inputs["out_b"], KT),
        "bff1": featmaj(inputs["ff1_b"], 32),
        "bff2": featmaj(inputs["ff2_b"], KT),
        "ln1w": featmaj(inputs["ln1_w"], KT),
        "ln1b": featmaj(inputs["ln1_b"], KT),
        "ln2w": featmaj(inputs["ln2_w"], KT),
        "ln2b": featmaj(inputs["ln2_b"], KT),
        "lnfw": featmaj(np.asarray(inputs["lnf_w"], dtype=f32).reshape(1, D), KT),
        "lnfb": featmaj(np.asarray(inputs["lnf_b"], dtype=f32).reshape(1, D), KT),
        "ones1": np.ones((P, 1), dtype=f32),
        "identb": np.eye(P, dtype=f32).astype(bf16),
    }

    x0_full = tok_emb[idx] + pos_emb[None, :, :]  # [B, T, D]

    embT = np.ascontiguousarray(tok_emb.T)  # [D, V]
    embT_pad = np.zeros((D, V_OFFS[-1] + VPAD), dtype=f32)
    embT_pad[:, :V] = embT
    lm_b = np.asarray(inputs["lm_head_b"], dtype=f32)
    blm_pad = np.full((V_OFFS[-1] + VPAD,), -1e30, dtype=f32)
    blm_pad[:V] = lm_b

    in_maps = []
    for c in range(8):
        g, s = divmod(c, 4)
        x0 = np.ascontiguousarray(x0_full[g, s * TS : (s + 1) * TS, :].T)
        tk = np.arange(T).reshape(4, 2, P)
        tq = s * TS + np.arange(TS)
        maskt = (tk[:, :, :, None] <= tq[None, None, None, :]).astype(bf16)
        maskt = np.ascontiguousarray(maskt.transpose(2, 0, 1, 3))
        m = dict(shared)
        m["x0"] = x0
        m["maskt"] = np.ascontiguousarray(maskt)
        m["embt"] = np.ascontiguousarray(embT_pad[:, V_OFFS[s] : V_OFFS[s] + VPAD])
        m["blm"] = np.broadcast_to(
            blm_pad[V_OFFS[s] : V_OFFS[s] + VPAD].astype(bf16), (P, VPAD)
        ).copy()
        in_maps.append(m)
    return in_maps, idx, targets


def kernel(**inputs):
    nc = _build_nc()
    in_maps, idx, targets = _prep_host(inputs)
    trace = _cache.get("trace", False)
    res = run_bass_kernel_spmd(nc, in_maps, list(range(8)), trace=trace)
    _cache["last_res"] = res

    logits = np.empty((B, T, V), dtype=np.float32)
    sumexp = np.zeros((B, T), dtype=np.float64)
    for c in range(8):
        g, s = divmod(c, 4)
        r = res.results[c]
        logits[g, :, V_OFFS[s] : V_OFFS[s] + V_LENS[s]] = r["logits"][:, : V_LENS[s]]
        sumexp[g] += r["sumexp"].reshape(T).astype(np.float64)

    logZ = np.log(sumexp)
    tgt = np.take_along_axis(
        logits.astype(np.float64),
        np.asarray(targets)[..., None].astype(np.int64),
        axis=2,
    )[..., 0]
    loss = np.float32(np.mean(logZ - tgt))
    return logits, loss


# revision 12
# speedup vs baseline: 1.2228x; 1.2228x over previous
"""GPT forward (L=8, D=1024, H=16, T=1024, B=2, V=50257) on 8 TRN2 NeuronCores.

Sharding: token-sharded layers (sequence parallel) — core c handles batch c//4,
token slice (c%4)*256..+256, weights replicated (bf16), one 4-core grouped
AllGather of (k, v^T) per layer for attention, vocab-sharded fp32r LM head
after an AllGather of the final layernormed activations. Residual stream fp32r,
matmul accumulation fp32 in PSUM.

kernel(**inputs) takes the FULL unsharded inputs (as from setup_inputs()) and
returns (logits [B,T,V] f32, loss scalar f32) like the reference.
"""

import sys

if "/opt/trn_rl_repo" not in sys.path:
    sys.path.insert(0, "/opt/trn_rl_repo")

import numpy as np
import ml_dtypes

import concourse.bass as bass
from concourse import bacc
import concourse.mybir as mybir
import concourse.tile as tile
from concourse.bass_utils import run_bass_kernel_spmd

V, D, H, L, T, B = 50257, 1024, 16, 8, 1024, 2
DH = D // H
DF = 4 * D
P = 128
TS = T // 4  # 256 tokens per core
KT = D // P  # 8
GROUPS = [[0, 1, 2, 3], [4, 5, 6, 7]]

VS0 = 12565
V_OFFS = [0, VS0, VS0 + 12564, VS0 + 2 * 12564]
V_LENS = [12565, 12564, 12564, 12564]
VPAD = 12800
VCH = VPAD // 512  # 25

F32 = mybir.dt.float32
F32R = mybir.dt.float32r
BF16 = mybir.dt.bfloat16

MGROUP = 4

_cache = {}


def _build_nc():
    if "nc" in _cache:
        return _cache["nc"]
    nc = bacc.Bacc()

    dp = nc.declare_dram_parameter
    x0_in = dp("x0", [D, TS], F32, isOutput=False)
    wqkv_in = dp("wqkv", [L, D, 3 * D], BF16, isOutput=False)
    wout_in = dp("wout", [L, D, D], BF16, isOutput=False)
    wff1_in = dp("wff1", [L, D, DF], BF16, isOutput=False)
    wff2_in = dp("wff2", [L, DF, D], BF16, isOutput=False)
    bqkv_in = dp("bqkv", [P, L * 24], F32, isOutput=False)
    bout_in = dp("bout", [P, L * KT], F32, isOutput=False)
    bff1_in = dp("bff1", [P, L * 32], F32, isOutput=False)
    bff2_in = dp("bff2", [P, L * KT], F32, isOutput=False)
    ln1w_in = dp("ln1w", [P, L * KT], F32, isOutput=False)
    ln1b_in = dp("ln1b", [P, L * KT], F32, isOutput=False)
    ln2w_in = dp("ln2w", [P, L * KT], F32, isOutput=False)
    ln2b_in = dp("ln2b", [P, L * KT], F32, isOutput=False)
    lnfw_in = dp("lnfw", [P, KT], F32, isOutput=False)
    lnfb_in = dp("lnfb", [P, KT], F32, isOutput=False)
    maskt_in = dp("maskt", [P, 4, 2, TS], BF16, isOutput=False)
    embt_in = dp("embt", [D, VPAD], F32, isOutput=False)
    blm_in = dp("blm", [P, VPAD], BF16, isOutput=False)  # pre-broadcast on host
    ones_in = dp("ones1", [P, 1], F32, isOutput=False)
    identb_in = dp("identb", [P, P], BF16, isOutput=False)

    logits_out = dp("logits", [T, VPAD], F32, isOutput=True)
    sumexp_out = dp("sumexp", [8, P], F32, isOutput=True)

    kv_cc_in = [nc.dram_tensor(f"kvin{l}", [2 * D, TS], BF16) for l in range(L)]
    kv_cc_out = [nc.dram_tensor(f"kvout{l}", [8 * D, TS], BF16) for l in range(L)]
    xl_cc_in = nc.dram_tensor("xlin", [D, TS], F32)
    xl_cc_out = nc.dram_tensor("xlout", [4 * D, TS], F32)

    eps = 1e-5

    with tile.TileContext(nc) as tc:
        with (
            tc.tile_pool(name="const", bufs=1) as constp,
            tc.tile_pool(name="acts", bufs=1) as acts,
            tc.tile_pool(name="small", bufs=2) as small,
            tc.tile_pool(name="drains", bufs=4) as drains,
            tc.tile_pool(name="ps_mm", bufs=MGROUP, space="PSUM") as ps_mm,
            tc.tile_pool(name="ps_sc", bufs=2, space="PSUM") as ps_sc,
            tc.tile_pool(name="ps_o", bufs=1, space="PSUM") as ps_o,
            tc.tile_pool(name="ps_sm", bufs=1, space="PSUM") as ps_sm,
        ):
            # ---- constants ----
            ones_r = constp.tile([P, 1], F32R)
            nc.sync.dma_start(ones_r[:], ones_in[:].bitcast(F32R))
            identb = constp.tile([P, P], BF16)
            nc.sync.dma_start(identb[:], identb_in[:])
            ones_b = constp.tile([P, 1], BF16)
            nc.vector.memset(ones_b[:], 1.0)
            maskt = constp.tile([P, 4, 2, TS], BF16)
            nc.sync.dma_start(maskt[:], maskt_in[:])
            params = {}
            for name, t_in, width in (
                ("ln1w", ln1w_in, KT),
                ("ln1b", ln1b_in, KT),
                ("ln2w", ln2w_in, KT),
                ("ln2b", ln2b_in, KT),
                ("bqkv", bqkv_in, 24),
                ("bout", bout_in, KT),
                ("bff1", bff1_in, 32),
                ("bff2", bff2_in, KT),
            ):
                tl = constp.tile([P, L * width], F32, tag=name, name=name + "_t")
                nc.sync.dma_start(tl[:], t_in[:])
                params[name] = (tl, width)
            lnfw_t = constp.tile([P, KT], F32)
            nc.sync.dma_start(lnfw_t[:], lnfw_in[:])
            lnfb_t = constp.tile([P, KT], F32)
            nc.sync.dma_start(lnfb_t[:], lnfb_in[:])

            def pslice(name, l):
                tl, width = params[name]
                return tl[:, l * width : (l + 1) * width]

            # ---- residual ----
            x = acts.tile([P, KT, TS], F32R)
            nc.sync.dma_start(
                x[:], x0_in[:].rearrange("(ko p) t -> p ko t", p=P).bitcast(F32R)
            )

            def make_layernorm(wrk):
                def layernorm_into(h_out, wvec, bvec):
                    xsq = wrk.tile([P, KT, TS], F32R, tag="xsq")
                    nc.scalar.activation(
                        xsq[:], x[:].bitcast(F32), mybir.ActivationFunctionType.Square
                    )
                    s1 = ps_sc.tile([1, TS], F32, tag="sc")
                    for k in range(KT):
                        nc.tensor.matmul(
                            s1[:], ones_r[:], x[:, k, :],
                            start=(k == 0), stop=(k == KT - 1),
                        )
                    s2 = ps_sc.tile([1, TS], F32, tag="sc")
                    for k in range(KT):
                        nc.tensor.matmul(
                            s2[:], ones_r[:], xsq[:, k, :],
                            start=(k == 0), stop=(k == KT - 1),
                        )
                    stat = small.tile([1, 4, TS], F32, tag="stat")
                    mean = stat[:, 0, :]
                    msq = stat[:, 1, :]
                    a_ = stat[:, 2, :]
                    b_ = stat[:, 3, :]
                    nc.vector.tensor_scalar_mul(mean, s1[:], 1.0 / D)
                    nc.vector.tensor_scalar_mul(msq, s2[:], 1.0 / D)
                    nc.vector.tensor_tensor(b_, mean, mean, op=mybir.AluOpType.mult)
                    nc.vector.tensor_tensor(a_, msq, b_, op=mybir.AluOpType.subtract)
                    nc.vector.tensor_scalar_add(a_, a_, eps)
                    nc.scalar.activation(a_, a_, mybir.ActivationFunctionType.Sqrt)
                    nc.vector.reciprocal(a_, a_)
                    nc.vector.tensor_tensor(b_, mean, a_, op=mybir.AluOpType.mult)
                    nc.vector.tensor_scalar_mul(b_, b_, -1.0)
                    ab_bc = small.tile([P, 2, TS], F32, tag="abbc")
                    nc.gpsimd.partition_broadcast(ab_bc[:, 0, :], a_)
                    nc.gpsimd.partition_broadcast(ab_bc[:, 1, :], b_)
                    tmp = wrk.tile([P, KT, TS], F32, tag="lntmp")
                    for k in range(KT):
                        nc.vector.tensor_tensor(
                            tmp[:, k, :], x[:, k, :].bitcast(F32), ab_bc[:, 0, :],
                            op=mybir.AluOpType.mult,
                        )
                        nc.vector.tensor_tensor(
                            tmp[:, k, :], tmp[:, k, :], ab_bc[:, 1, :],
                            op=mybir.AluOpType.add,
                        )
                        nc.scalar.activation(
                            h_out[:, k, :], tmp[:, k, :],
                            mybir.ActivationFunctionType.Identity,
                            bias=bvec[:, k : k + 1], scale=wvec[:, k : k + 1],
                        )

                return layernorm_into

            def make_matmul_block(wstream):
                def matmul_block(out_cb, w_dram, KTILES, MT, rhs, l):
                    n_mg = (MT + MGROUP - 1) // MGROUP
                    for mg in range(n_mg):
                        m0 = mg * MGROUP
                        m1 = min(m0 + MGROUP, MT)
                        nm = m1 - m0
                        pts = [ps_mm.tile([P, TS], F32, tag="mm", name=f"mmps{mg}_{i}") for i in range(nm)]
                        # stream weights in chunks of up to 8 k-tiles, one DMA each
                        wts = {}
                        for kc in range(0, KTILES, 8):
                            nk = min(8, KTILES - kc)
                            wt = wstream.tile([P, 8, MGROUP * P], BF16, tag="w",
                                              name=f"wt{mg}_{kc}")
                            nc.sync.dma_start(
                                wt[:, :nk, : nm * P],
                                w_dram[l, kc * P : (kc + nk) * P, m0 * P : m1 * P]
                                .rearrange("(ko p) m -> p ko m", p=P),
                            )
                            wts[kc] = wt
                        for k in range(KTILES):
                            wt = wts[(k // 8) * 8]
                            ks = k % 8
                            for mi in range(nm):
                                nc.tensor.matmul(
                                    pts[mi][:],
                                    wt[:, ks, mi * P : (mi + 1) * P],
                                    rhs[:, k, :],
                                    start=(k == 0),
                                    stop=(k == KTILES - 1),
                                )
                        for mi in range(nm):
                            out_cb(m0 + mi, pts[mi])

                return matmul_block

            # ================= layers (own pool scope) =================
            with (
                tc.tile_pool(name="wrk", bufs=1) as wrk,
                tc.tile_pool(name="wstream", bufs=3) as wstream,
                tc.tile_pool(name="probs", bufs=2) as probsp,
            ):
                layernorm_into = make_layernorm(wrk)
                matmul_block = make_matmul_block(wstream)

                for l in range(L):
                    # ---------------- LN1 -> h ----------------
                    h = wrk.tile([P, KT, TS], BF16, tag="h")
                    layernorm_into(h, pslice("ln1w", l), pslice("ln1b", l))

                    # ---------------- qkv ----------------
                    qkv = wrk.tile([P, 24, TS], BF16, tag="qkv")
                    bq = params["bqkv"][0]

                    def qkv_drain(m, pt, l=l, qkv=qkv, bq=bq):
                        nc.scalar.activation(
                            qkv[:, m, :], pt[:],
                            mybir.ActivationFunctionType.Identity,
                            bias=bq[:, l * 24 + m : l * 24 + m + 1],
                        )

                    matmul_block(qkv_drain, wqkv_in, KT, 24, h, l)

                    # ------------- pack (k, v^T), AllGather -------------
                    nc.sync.dma_start(
                        kv_cc_in[l][0:D, :].rearrange("(o p) t -> p o t", p=P),
                        qkv[:, 8:16, :],
                    )
                    vt_mine = wrk.tile([P, 2, D], BF16, tag="vtm")
                    for dt in range(KT):
                        for tt in range(2):
                            ptp = ps_sm.tile([P, P], BF16, tag="small")
                            nc.tensor.transpose(
                                ptp[:], qkv[:, 16 + dt, tt * P : (tt + 1) * P],
                                identb[:],
                            )
                            nc.vector.tensor_copy(
                                out=vt_mine[:, tt, dt * P : (dt + 1) * P], in_=ptp[:]
                            )
                    nc.sync.dma_start(
                        kv_cc_in[l][D : 2 * D, :]
                        .rearrange("(a b) c -> a (b c)", a=TS)
                        .rearrange("(tt p) d -> p tt d", p=P),
                        vt_mine[:],
                    )
                    nc.gpsimd.collective_compute(
                        "AllGather",
                        mybir.AluOpType.bypass,
                        replica_groups=GROUPS,
                        ins=[kv_cc_in[l][:]],
                        outs=[kv_cc_out[l][:]],
                    )
                    kg = wrk.tile([P, KT, 4, TS], BF16, tag="kg")
                    vt_pad = wrk.tile([P, 2, 4, H, 66], BF16, tag="vtp")
                    nc.vector.memset(vt_pad[:, :, :, :, 64:65], 1.0)
                    for r in range(4):
                        base = r * 2 * D
                        nc.sync.dma_start(
                            kg[:, :, r, :],
                            kv_cc_out[l][base : base + D, :].rearrange(
                                "(o p) t -> p o t", p=P
                            ),
                        )
                        vsrc = (
                            kv_cc_out[l][base + D : base + 2 * D, :]
                            .rearrange("(a b) c -> a (b c)", a=TS)
                            .rearrange("(tt p) (hh d) -> tt p hh d", p=P, d=64)
                        )
                        for tt in range(2):
                            nc.sync.dma_start(
                                vt_pad[:, tt, r, :, 0:64], vsrc[tt]
                            )

                    # ---------------- attention ----------------
                    o = wrk.tile([P, KT, TS], BF16, tag="o")
                    for hh in range(H):
                        ph = (hh % 2) * 64
                        dt = hh // 2
                        probs = probsp.tile([P, 8, TS], BF16, tag="pr")
                        for r in range(4):
                            for tt in range(2):
                                rj = r * 2 + tt
                                psc = ps_sc.tile([P, TS], F32, tag="sc")
                                nc.tensor.matmul(
                                    psc[:],
                                    kg[ph : ph + 64, dt, r, tt * P : (tt + 1) * P],
                                    qkv[ph : ph + 64, dt, :],
                                    start=True,
                                    stop=True,
                                )
                                nc.scalar.activation(
                                    probs[:, rj, :], psc[:],
                                    mybir.ActivationFunctionType.Exp,
                                )
                                nc.vector.tensor_tensor(
                                    probs[:, rj, :], probs[:, rj, :],
                                    maskt[:, r, tt, :],
                                    op=mybir.AluOpType.mult,
                                )
                        po = ps_o.tile([65, TS], F32, tag="o")
                        for r in range(4):
                            for tt in range(2):
                                rj = r * 2 + tt
                                nc.tensor.matmul(
                                    po[:],
                                    vt_pad[:, tt, r, hh, 0:65],
                                    probs[:, rj, :],
                                    start=(rj == 0),
                                    stop=(rj == 7),
                                )
                        rec_hi = small.tile([P, TS], F32, tag="rechi")
                        nc.vector.reciprocal(rec_hi[64:65, :], po[64:65, :])
                        rec0 = small.tile([1, TS], F32, tag="rec")
                        nc.sync.dma_start(rec0[:], rec_hi[64:65, :])
                        rec_bc = small.tile([64, TS], F32, tag="recbc")
                        nc.gpsimd.partition_broadcast(rec_bc[:], rec0[:])
                        o_tmp = drains.tile([64, TS], BF16, tag="otmp")
                        nc.vector.tensor_tensor(
                            o_tmp[:], po[0:64, :], rec_bc[:],
                            op=mybir.AluOpType.mult,
                        )
                        nc.sync.dma_start(o[ph : ph + 64, dt, :], o_tmp[:])

                    # ---------------- out proj + residual ----------------
                    def out_drain(m, pt, l=l):
                        tmp = drains.tile([P, TS], F32, tag="ytmp")
                        nc.scalar.activation(
                            tmp[:], pt[:], mybir.ActivationFunctionType.Identity,
                            bias=pslice("bout", l)[:, m : m + 1],
                        )
                        nc.vector.tensor_tensor(
                            x[:, m, :], x[:, m, :].bitcast(F32), tmp[:],
                            op=mybir.AluOpType.add,
                        )

                    matmul_block(out_drain, wout_in, KT, KT, o, l)

                    # ---------------- LN2 -> h2, ff1 ----------------
                    h2 = wrk.tile([P, KT, TS], BF16, tag="h")
                    layernorm_into(h2, pslice("ln2w", l), pslice("ln2b", l))
                    ff = wrk.tile([P, 32, TS], BF16, tag="qkv")

                    def ff1_drain(m, pt, l=l, ff=ff):
                        nc.scalar.activation(
                            ff[:, m, :], pt[:], mybir.ActivationFunctionType.Gelu,
                            bias=pslice("bff1", l)[:, m : m + 1],
                        )

                    matmul_block(ff1_drain, wff1_in, KT, 32, h2, l)

                    # ---------------- ff2 + residual ----------------
                    def ff2_drain(m, pt, l=l):
                        tmp = drains.tile([P, TS], F32, tag="ytmp")
                        nc.scalar.activation(
                            tmp[:], pt[:], mybir.ActivationFunctionType.Identity,
                            bias=pslice("bff2", l)[:, m : m + 1],
                        )
                        nc.vector.tensor_tensor(
                            x[:, m, :], x[:, m, :].bitcast(F32), tmp[:],
                            op=mybir.AluOpType.add,
                        )

                    matmul_block(ff2_drain, wff2_in, 32, KT, ff, l)

                # ---- final LN + AllGather x_lnf ----
                xl = wrk.tile([P, KT, TS], F32R, tag="h")
                layernorm_into(xl, lnfw_t[:], lnfb_t[:])
                nc.sync.dma_start(
                    xl_cc_in[:].rearrange("(o p) t -> p o t", p=P),
                    xl[:].bitcast(F32),
                )
                nc.gpsimd.collective_compute(
                    "AllGather",
                    mybir.AluOpType.bypass,
                    replica_groups=GROUPS,
                    ins=[xl_cc_in[:]],
                    outs=[xl_cc_out[:]],
                )

            # ================= LM head (own pool scope) =================
            with tc.tile_pool(name="lmp", bufs=1) as lmp, tc.tile_pool(
                name="embs", bufs=2
            ) as embs:
                xlg = lmp.tile([P, 4, KT, TS], F32R)
                nc.sync.dma_start(
                    xlg[:],
                    xl_cc_out[:]
                    .rearrange("(r o p) t -> p r o t", p=P, o=KT)
                    .bitcast(F32R),
                )
                bias_bc = lmp.tile([P, VPAD], BF16)
                nc.sync.dma_start(bias_bc[:], blm_in[:])
                sump = lmp.tile([P, 8, VCH], F32)
                for vc in range(VCH):
                    et = embs.tile([P, KT, 512], F32R, tag="emb")
                    nc.sync.dma_start(
                        et[:],
                        embt_in[:, vc * 512 : (vc + 1) * 512]
                        .rearrange("(o p) c -> p o c", p=P)
                        .bitcast(F32R),
                    )
                    for r in range(4):
                        for tt in range(2):
                            ti = r * 2 + tt
                            pl = ps_mm.tile([P, 512], F32, tag="mm")
                            for k in range(KT):
                                nc.tensor.matmul(
                                    pl[:],
                                    xlg[:, r, k, tt * P : (tt + 1) * P],
                                    et[:, k, :],
                                    start=(k == 0),
                                    stop=(k == KT - 1),
                                )
                            lsb = drains.tile([P, 512], F32, tag="lg")
                            nc.vector.tensor_tensor(
                                lsb[:], pl[:],
                                bias_bc[:, vc * 512 : (vc + 1) * 512],
                                op=mybir.AluOpType.add,
                            )
                            nc.sync.dma_start(
                                logits_out[
                                    ti * P : (ti + 1) * P, vc * 512 : (vc + 1) * 512
                                ],
                                lsb[:],
                            )
                            esc = drains.tile([P, 512], F32, tag="esc")
                            nc.scalar.activation(
                                esc[:], lsb[:], mybir.ActivationFunctionType.Exp,
                                accum_out=sump[:, ti, vc : vc + 1],
                            )
                stile = lmp.tile([P, 8], F32)
                for ti in range(8):
                    nc.vector.tensor_reduce(
                        stile[:, ti : ti + 1], sump[:, ti, :],
                        mybir.AxisListType.X, mybir.AluOpType.add,
                    )
                nc.sync.dma_start(sumexp_out[:].rearrange("a p -> p a"), stile[:])

    nc.finalize()
    _cache["nc"] = nc
    return nc


def _prep_host(inputs):
    f32 = np.float32
    bf16 = ml_dtypes.bfloat16
    idx = np.asarray(inputs["idx"])
    targets = np.asarray(inputs["targets"])
    tok_emb = np.asarray(inputs["tok_emb"], dtype=f32)
    pos_emb = np.asarray(inputs["pos_emb"], dtype=f32)

    qkv_wT = np.ascontiguousarray(
        np.transpose(np.asarray(inputs["qkv_w"], dtype=f32), (0, 2, 1))
    )
    qkv_wT[:, :, :D] *= f32(1.0 / np.sqrt(DH))
    bqkv = np.array(np.asarray(inputs["qkv_b"], dtype=f32))
    bqkv[:, :D] *= f32(1.0 / np.sqrt(DH))
    def featmaj(a, width):
        # [L, M] -> [P, L*width] where column l*width+mo holds a[l, mo*128+p]
        a = np.asarray(a, dtype=f32)
        Lw = a.shape[0]
        out = a.reshape(Lw, width, P).transpose(2, 0, 1).reshape(P, Lw * width)
        return np.ascontiguousarray(out)

    shared = {
        "wqkv": qkv_wT.astype(bf16),
        "wout": np.ascontiguousarray(
            np.transpose(np.asarray(inputs["out_w"], dtype=f32), (0, 2, 1))
        ).astype(bf16),
        "wff1": np.ascontiguousarray(
            np.transpose(np.asarray(inputs["ff1_w"], dtype=f32), (0, 2, 1))
        ).astype(bf16),
        "wff2": np.ascontiguousarray(
            np.transpose(np.asarray(inputs["ff2_w"], dtype=f32), (0, 2, 1))
        ).astype(bf16),
        "bqkv": featmaj(bqkv, 24),
        "bout": featmaj(inputs["out_b"], KT),
        "bff1": featmaj(inputs["ff1_b"], 32),
        "bff2": featmaj(inputs["ff2_b"], KT),
        "ln1w": featmaj(inputs["ln1_w"], KT),
        "ln1b": featmaj(inputs["ln1_b"], KT),
        "ln2w": featmaj(inputs["ln2_w"], KT),
        "ln2b": featmaj(inputs["ln2_b"], KT),
        "lnfw": featmaj(np.asarray(inputs["lnf_w"], dtype=f32).reshape(1, D), KT),
        "lnfb": featmaj(np.asarray(inputs["lnf_b"], dtype=f32).reshape(1, D), KT),
        "ones1": np.ones((P, 1), dtype=f32),
        "identb": np.eye(P, dtype=f32).astype(bf16),
    }

    x0_full = tok_emb[idx] + pos_emb[None, :, :]  # [B, T, D]

    embT = np.ascontiguousarray(tok_emb.T)  # [D, V]
    embT_pad = np.zeros((D, V_OFFS[-1] + VPAD), dtype=f32)
    embT_pad[:, :V] = embT
    lm_b = np.asarray(inputs["lm_head_b"], dtype=f32)
    blm_pad = np.full((V_OFFS[-1] + VPAD,), -1e30, dtype=f32)
    blm_pad[:V] = lm_b

    in_maps = []
    for c in range(8):
        g, s = divmod(c, 4)
        x0 = np.ascontiguousarray(x0_full[g, s * TS : (s + 1) * TS, :].T)
        tk = np.arange(T).reshape(4, 2, P)
        tq = s * TS + np.arange(TS)
        maskt = (tk[:, :, :, None] <= tq[None, None, None, :]).astype(bf16)
        maskt = np.ascontiguousarray(maskt.transpose(2, 0, 1, 3))
        m = dict(shared)
        m["x0"] = x0
        m["maskt"] = np.ascontiguousarray(maskt)
        m["embt"] = np.ascontiguousarray(embT_pad[:, V_OFFS[s] : V_OFFS[s] + VPAD])
        m["blm"] = np.broadcast_to(
            blm_pad[V_OFFS[s] : V_OFFS[s] + VPAD].astype(bf16), (P, VPAD)
        ).copy()
        in_maps.append(m)
    return in_maps, idx, targets


def kernel(**inputs):
    nc = _build_nc()
    in_maps, idx, targets = _prep_host(inputs)
    trace = _cache.get("trace", False)
    res = run_bass_kernel_spmd(nc, in_maps, list(range(8)), trace=trace)
    _cache["last_res"] = res

    logits = np.empty((B, T, V), dtype=np.float32)
    sumexp = np.zeros((B, T), dtype=np.float64)
    for c in range(8):
        g, s = divmod(c, 4)
        r = res.results[c]
        logits[g, :, V_OFFS[s] : V_OFFS[s] + V_LENS[s]] = r["logits"][:, : V_LENS[s]]
        sumexp[g] += r["sumexp"].reshape(T).astype(np.float64)

    logZ = np.log(sumexp)
    tgt = np.take_along_axis(
        logits.astype(np.float64),
        np.asarray(targets)[..., None].astype(np.int64),
        axis=2,
    )[..., 0]
    loss = np.float32(np.mean(logZ - tgt))
    return logits, loss


# revision 13
# speedup vs baseline: 1.3661x; 1.1171x over previous
"""GPT forward (L=8, D=1024, H=16, T=1024, B=2, V=50257) on 8 TRN2 NeuronCores.

Sharding: token-sharded layers (sequence parallel) — core c handles batch c//4,
token slice (c%4)*256..+256, weights replicated (bf16), one 4-core grouped
AllGather of (k, v^T) per layer for attention, vocab-sharded fp32r LM head
after an AllGather of the final layernormed activations. Residual stream fp32r,
matmul accumulation fp32 in PSUM.

kernel(**inputs) takes the FULL unsharded inputs (as from setup_inputs()) and
returns (logits [B,T,V] f32, loss scalar f32) like the reference.
"""

import sys

if "/opt/trn_rl_repo" not in sys.path:
    sys.path.insert(0, "/opt/trn_rl_repo")

import numpy as np
import ml_dtypes

import concourse.bass as bass
from concourse import bacc
import concourse.mybir as mybir
import concourse.tile as tile
from concourse.bass_utils import run_bass_kernel_spmd

V, D, H, L, T, B = 50257, 1024, 16, 8, 1024, 2
DH = D // H
DF = 4 * D
P = 128
TS = T // 4  # 256 tokens per core
KT = D // P  # 8
GROUPS = [[0, 1, 2, 3], [4, 5, 6, 7]]

VS0 = 12565
V_OFFS = [0, VS0, VS0 + 12564, VS0 + 2 * 12564]
V_LENS = [12565, 12564, 12564, 12564]
VPAD = 12800
VCH = VPAD // 512  # 25

F32 = mybir.dt.float32
F32R = mybir.dt.float32r
BF16 = mybir.dt.bfloat16

MGROUP = 4

_cache = {}


def _build_nc():
    if "nc" in _cache:
        return _cache["nc"]
    nc = bacc.Bacc()

    dp = nc.declare_dram_parameter
    x0_in = dp("x0", [D, TS], F32, isOutput=False)
    wqkv_in = dp("wqkv", [L, D, 3 * D], BF16, isOutput=False)
    wout_in = dp("wout", [L, D, D], BF16, isOutput=False)
    wff1_in = dp("wff1", [L, D, DF], BF16, isOutput=False)
    wff2_in = dp("wff2", [L, DF, D], BF16, isOutput=False)
    bqkv_in = dp("bqkv", [P, L * 24], F32, isOutput=False)
    bout_in = dp("bout", [P, L * KT], F32, isOutput=False)
    bff1_in = dp("bff1", [P, L * 32], F32, isOutput=False)
    bff2_in = dp("bff2", [P, L * KT], F32, isOutput=False)
    ln1w_in = dp("ln1w", [P, L * KT], F32, isOutput=False)
    ln1b_in = dp("ln1b", [P, L * KT], F32, isOutput=False)
    ln2w_in = dp("ln2w", [P, L * KT], F32, isOutput=False)
    ln2b_in = dp("ln2b", [P, L * KT], F32, isOutput=False)
    lnfw_in = dp("lnfw", [P, KT], F32, isOutput=False)
    lnfb_in = dp("lnfb", [P, KT], F32, isOutput=False)
    maskt_in = dp("maskt", [P, 4, 2, TS], BF16, isOutput=False)
    embt_in = dp("embt", [D, VPAD], F32, isOutput=False)
    blm_in = dp("blm", [P, VPAD], BF16, isOutput=False)  # pre-broadcast on host
    ones_in = dp("ones1", [P, 1], F32, isOutput=False)
    identb_in = dp("identb", [P, P], BF16, isOutput=False)

    logits_out = dp("logits", [T, VPAD], F32, isOutput=True)
    sumexp_out = dp("sumexp", [8, P], F32, isOutput=True)

    kv_cc_in = [nc.dram_tensor(f"kvin{l}", [2 * D, TS], BF16) for l in range(L)]
    kv_cc_out = [nc.dram_tensor(f"kvout{l}", [8 * D, TS], BF16) for l in range(L)]
    xl_cc_in = nc.dram_tensor("xlin", [D, TS], F32)
    xl_cc_out = nc.dram_tensor("xlout", [4 * D, TS], F32)

    eps = 1e-5

    with tile.TileContext(nc) as tc:
        with (
            tc.tile_pool(name="const", bufs=1) as constp,
            tc.tile_pool(name="acts", bufs=1) as acts,
            tc.tile_pool(name="small", bufs=2) as small,
            tc.tile_pool(name="drains", bufs=4) as drains,
            tc.tile_pool(name="ps_mm", bufs=MGROUP, space="PSUM") as ps_mm,
            tc.tile_pool(name="ps_sc", bufs=2, space="PSUM") as ps_sc,
            tc.tile_pool(name="ps_o", bufs=1, space="PSUM") as ps_o,
            tc.tile_pool(name="ps_sm", bufs=1, space="PSUM") as ps_sm,
        ):
            # ---- constants ----
            ones_r = constp.tile([P, 1], F32R)
            nc.sync.dma_start(ones_r[:], ones_in[:].bitcast(F32R))
            identb = constp.tile([P, P], BF16)
            nc.sync.dma_start(identb[:], identb_in[:])
            ones_b = constp.tile([P, 1], BF16)
            nc.vector.memset(ones_b[:], 1.0)
            maskt = constp.tile([P, 4, 2, TS], BF16)
            nc.sync.dma_start(maskt[:], maskt_in[:])
            params = {}
            for name, t_in, width in (
                ("ln1w", ln1w_in, KT),
                ("ln1b", ln1b_in, KT),
                ("ln2w", ln2w_in, KT),
                ("ln2b", ln2b_in, KT),
                ("bqkv", bqkv_in, 24),
                ("bout", bout_in, KT),
                ("bff1", bff1_in, 32),
                ("bff2", bff2_in, KT),
            ):
                tl = constp.tile([P, L * width], F32, tag=name, name=name + "_t")
                nc.sync.dma_start(tl[:], t_in[:])
                params[name] = (tl, width)
            lnfw_t = constp.tile([P, KT], F32)
            nc.sync.dma_start(lnfw_t[:], lnfw_in[:])
            lnfb_t = constp.tile([P, KT], F32)
            nc.sync.dma_start(lnfb_t[:], lnfb_in[:])

            def pslice(name, l):
                tl, width = params[name]
                return tl[:, l * width : (l + 1) * width]

            # ---- residual ----
            x = acts.tile([P, KT, TS], F32R)
            nc.sync.dma_start(
                x[:], x0_in[:].rearrange("(ko p) t -> p ko t", p=P).bitcast(F32R)
            )

            def make_layernorm(wrk):
                def layernorm_into(h_out, wvec, bvec):
                    xsq = wrk.tile([P, KT, TS], F32R, tag="xsq")
                    nc.scalar.activation(
                        xsq[:], x[:].bitcast(F32), mybir.ActivationFunctionType.Square
                    )
                    s1 = ps_sc.tile([1, TS], F32, tag="sc")
                    for k in range(KT):
                        nc.tensor.matmul(
                            s1[:], ones_r[:], x[:, k, :],
                            start=(k == 0), stop=(k == KT - 1),
                        )
                    s2 = ps_sc.tile([1, TS], F32, tag="sc")
                    for k in range(KT):
                        nc.tensor.matmul(
                            s2[:], ones_r[:], xsq[:, k, :],
                            start=(k == 0), stop=(k == KT - 1),
                        )
                    stat = small.tile([1, 4, TS], F32, tag="stat")
                    mean = stat[:, 0, :]
                    msq = stat[:, 1, :]
                    a_ = stat[:, 2, :]
                    b_ = stat[:, 3, :]
                    nc.vector.tensor_scalar_mul(mean, s1[:], 1.0 / D)
                    nc.vector.tensor_scalar_mul(msq, s2[:], 1.0 / D)
                    nc.vector.tensor_tensor(b_, mean, mean, op=mybir.AluOpType.mult)
                    nc.vector.tensor_tensor(a_, msq, b_, op=mybir.AluOpType.subtract)
                    nc.vector.tensor_scalar_add(a_, a_, eps)
                    nc.scalar.activation(a_, a_, mybir.ActivationFunctionType.Sqrt)
                    nc.vector.reciprocal(a_, a_)
                    nc.vector.tensor_tensor(b_, mean, a_, op=mybir.AluOpType.mult)
                    nc.vector.tensor_scalar_mul(b_, b_, -1.0)
                    ab_bc = small.tile([P, 2, TS], F32, tag="abbc")
                    nc.gpsimd.partition_broadcast(ab_bc[:, 0, :], a_)
                    nc.gpsimd.partition_broadcast(ab_bc[:, 1, :], b_)
                    tmp = wrk.tile([P, KT, TS], F32, tag="lntmp")
                    for k in range(KT):
                        nc.vector.tensor_tensor(
                            tmp[:, k, :], x[:, k, :].bitcast(F32), ab_bc[:, 0, :],
                            op=mybir.AluOpType.mult,
                        )
                        nc.vector.tensor_tensor(
                            tmp[:, k, :], tmp[:, k, :], ab_bc[:, 1, :],
                            op=mybir.AluOpType.add,
                        )
                        nc.scalar.activation(
                            h_out[:, k, :], tmp[:, k, :],
                            mybir.ActivationFunctionType.Identity,
                            bias=bvec[:, k : k + 1], scale=wvec[:, k : k + 1],
                        )

                return layernorm_into

            def make_matmul_block(wstream):
                def matmul_block(out_cb, w_dram, KTILES, MT, rhs, l, m_start=0):
                    n_mg = (MT + MGROUP - 1) // MGROUP
                    for mg in range(n_mg):
                        m0 = m_start + mg * MGROUP
                        m1 = min(m0 + MGROUP, m_start + MT)
                        nm = m1 - m0
                        pts = [ps_mm.tile([P, TS], F32, tag="mm", name=f"mmps{mg}_{i}") for i in range(nm)]
                        # stream weights in chunks of up to 8 k-tiles, one DMA each
                        wts = {}
                        for kc in range(0, KTILES, 8):
                            nk = min(8, KTILES - kc)
                            wt = wstream.tile([P, 8, MGROUP * P], BF16, tag="w",
                                              name=f"wt{mg}_{kc}")
                            nc.sync.dma_start(
                                wt[:, :nk, : nm * P],
                                w_dram[l, kc * P : (kc + nk) * P, m0 * P : m1 * P]
                                .rearrange("(ko p) m -> p ko m", p=P),
                            )
                            wts[kc] = wt
                        for k in range(KTILES):
                            wt = wts[(k // 8) * 8]
                            ks = k % 8
                            for mi in range(nm):
                                nc.tensor.matmul(
                                    pts[mi][:],
                                    wt[:, ks, mi * P : (mi + 1) * P],
                                    rhs[:, k, :],
                                    start=(k == 0),
                                    stop=(k == KTILES - 1),
                                )
                        for mi in range(nm):
                            out_cb(m0 + mi, pts[mi])

                return matmul_block

            # ================= layers (own pool scope) =================
            with (
                tc.tile_pool(name="wrk", bufs=1) as wrk,
                tc.tile_pool(name="wstream", bufs=3) as wstream,
                tc.tile_pool(name="probs", bufs=2) as probsp,
            ):
                layernorm_into = make_layernorm(wrk)
                matmul_block = make_matmul_block(wstream)

                for l in range(L):
                    # ---------------- LN1 -> h ----------------
                    h = wrk.tile([P, KT, TS], BF16, tag="h")
                    layernorm_into(h, pslice("ln1w", l), pslice("ln1b", l))

                    # ---------------- qkv ----------------
                    qkv = wrk.tile([P, 24, TS], BF16, tag="qkv")
                    bq = params["bqkv"][0]

                    def qkv_drain(m, pt, l=l, qkv=qkv, bq=bq):
                        nc.scalar.activation(
                            qkv[:, m, :], pt[:],
                            mybir.ActivationFunctionType.Identity,
                            bias=bq[:, l * 24 + m : l * 24 + m + 1],
                        )

                    matmul_block(qkv_drain, wqkv_in, KT, 16, h, l, m_start=8)

                    # ------------- pack (k, v^T), AllGather -------------
                    nc.sync.dma_start(
                        kv_cc_in[l][0:D, :].rearrange("(o p) t -> p o t", p=P),
                        qkv[:, 8:16, :],
                    )
                    vt_mine = wrk.tile([P, 2, D], BF16, tag="vtm")
                    for dt in range(KT):
                        for tt in range(2):
                            ptp = ps_sm.tile([P, P], BF16, tag="small")
                            nc.tensor.transpose(
                                ptp[:], qkv[:, 16 + dt, tt * P : (tt + 1) * P],
                                identb[:],
                            )
                            nc.vector.tensor_copy(
                                out=vt_mine[:, tt, dt * P : (dt + 1) * P], in_=ptp[:]
                            )
                    nc.sync.dma_start(
                        kv_cc_in[l][D : 2 * D, :]
                        .rearrange("(a b) c -> a (b c)", a=TS)
                        .rearrange("(tt p) d -> p tt d", p=P),
                        vt_mine[:],
                    )
                    nc.gpsimd.collective_compute(
                        "AllGather",
                        mybir.AluOpType.bypass,
                        replica_groups=GROUPS,
                        ins=[kv_cc_in[l][:]],
                        outs=[kv_cc_out[l][:]],
                    )
                    # q projection overlaps the AllGather
                    matmul_block(qkv_drain, wqkv_in, KT, 8, h, l, m_start=0)
                    kg = wrk.tile([P, KT, 4, TS], BF16, tag="kg")
                    vt_pad = wrk.tile([P, 2, 4, H, 66], BF16, tag="vtp")
                    nc.vector.memset(vt_pad[:, :, :, :, 64:65], 1.0)
                    for r in range(4):
                        base = r * 2 * D
                        nc.sync.dma_start(
                            kg[:, :, r, :],
                            kv_cc_out[l][base : base + D, :].rearrange(
                                "(o p) t -> p o t", p=P
                            ),
                        )
                        vsrc = (
                            kv_cc_out[l][base + D : base + 2 * D, :]
                            .rearrange("(a b) c -> a (b c)", a=TS)
                            .rearrange("(tt p) (hh d) -> tt p hh d", p=P, d=64)
                        )
                        for tt in range(2):
                            nc.sync.dma_start(
                                vt_pad[:, tt, r, :, 0:64], vsrc[tt]
                            )

                    # ---------------- attention ----------------
                    o = wrk.tile([P, KT, TS], BF16, tag="o")
                    for hh in range(H):
                        ph = (hh % 2) * 64
                        dt = hh // 2
                        probs = probsp.tile([P, 8, TS], BF16, tag="pr")
                        for r in range(4):
                            psc = ps_sc.tile([P, 2, TS], F32, tag="sc")
                            for tt in range(2):
                                nc.tensor.matmul(
                                    psc[:, tt, :],
                                    kg[ph : ph + 64, dt, r, tt * P : (tt + 1) * P],
                                    qkv[ph : ph + 64, dt, :],
                                    start=True,
                                    stop=True,
                                )
                            rj = r * 2
                            nc.scalar.activation(
                                probs[:, rj : rj + 2, :], psc[:],
                                mybir.ActivationFunctionType.Exp,
                            )
                            nc.vector.tensor_tensor(
                                probs[:, rj : rj + 2, :], probs[:, rj : rj + 2, :],
                                maskt[:, r, :, :],
                                op=mybir.AluOpType.mult,
                            )
                        po = ps_o.tile([65, TS], F32, tag="o")
                        for r in range(4):
                            for tt in range(2):
                                rj = r * 2 + tt
                                nc.tensor.matmul(
                                    po[:],
                                    vt_pad[:, tt, r, hh, 0:65],
                                    probs[:, rj, :],
                                    start=(rj == 0),
                                    stop=(rj == 7),
                                )
                        rec_hi = small.tile([P, TS], F32, tag="rechi")
                        nc.vector.reciprocal(rec_hi[64:65, :], po[64:65, :])
                        rec0 = small.tile([1, TS], F32, tag="rec")
                        nc.sync.dma_start(rec0[:], rec_hi[64:65, :])
                        rec_bc = small.tile([64, TS], F32, tag="recbc")
                        nc.gpsimd.partition_broadcast(rec_bc[:], rec0[:])
                        o_tmp = drains.tile([64, TS], BF16, tag="otmp")
                        nc.vector.tensor_tensor(
                            o_tmp[:], po[0:64, :], rec_bc[:],
                            op=mybir.AluOpType.mult,
                        )
                        nc.sync.dma_start(o[ph : ph + 64, dt, :], o_tmp[:])

                    # ---------------- out proj + residual ----------------
                    def out_drain(m, pt, l=l):
                        tmp = drains.tile([P, TS], F32, tag="ytmp")
                        nc.scalar.activation(
                            tmp[:], pt[:], mybir.ActivationFunctionType.Identity,
                            bias=pslice("bout", l)[:, m : m + 1],
                        )
                        nc.vector.tensor_tensor(
                            x[:, m, :], x[:, m, :].bitcast(F32), tmp[:],
                            op=mybir.AluOpType.add,
                        )

                    matmul_block(out_drain, wout_in, KT, KT, o, l)

                    # ---------------- LN2 -> h2, ff1 ----------------
                    h2 = wrk.tile([P, KT, TS], BF16, tag="h")
                    layernorm_into(h2, pslice("ln2w", l), pslice("ln2b", l))
                    ff = wrk.tile([P, 32, TS], BF16, tag="qkv")

                    def ff1_drain(m, pt, l=l, ff=ff):
                        nc.scalar.activation(
                            ff[:, m, :], pt[:], mybir.ActivationFunctionType.Gelu,
                            bias=pslice("bff1", l)[:, m : m + 1],
                        )

                    matmul_block(ff1_drain, wff1_in, KT, 32, h2, l)

                    # ---------------- ff2 + residual ----------------
                    def ff2_drain(m, pt, l=l):
                        tmp = drains.tile([P, TS], F32, tag="ytmp")
                        nc.scalar.activation(
                            tmp[:], pt[:], mybir.ActivationFunctionType.Identity,
                            bias=pslice("bff2", l)[:, m : m + 1],
                        )
                        nc.vector.tensor_tensor(
                            x[:, m, :], x[:, m, :].bitcast(F32), tmp[:],
                            op=mybir.AluOpType.add,
                        )

                    matmul_block(ff2_drain, wff2_in, 32, KT, ff, l)

                # ---- final LN + AllGather x_lnf ----
                xl = wrk.tile([P, KT, TS], F32R, tag="h")
                layernorm_into(xl, lnfw_t[:], lnfb_t[:])
                nc.sync.dma_start(
                    xl_cc_in[:].rearrange("(o p) t -> p o t", p=P),
                    xl[:].bitcast(F32),
                )
                nc.gpsimd.collective_compute(
                    "AllGather",
                    mybir.AluOpType.bypass,
                    replica_groups=GROUPS,
                    ins=[xl_cc_in[:]],
                    outs=[xl_cc_out[:]],
                )

            # ================= LM head (own pool scope) =================
            with tc.tile_pool(name="lmp", bufs=1) as lmp, tc.tile_pool(
                name="embs", bufs=2
            ) as embs:
                xlg = lmp.tile([P, 4, KT, TS], F32R)
                nc.sync.dma_start(
                    xlg[:],
                    xl_cc_out[:]
                    .rearrange("(r o p) t -> p r o t", p=P, o=KT)
                    .bitcast(F32R),
                )
                bias_bc = lmp.tile([P, VPAD], BF16)
                nc.sync.dma_start(bias_bc[:], blm_in[:])
                sump = lmp.tile([P, 8, VCH], F32)
                for vc in range(VCH):
                    et = embs.tile([P, KT, 512], F32R, tag="emb")
                    nc.sync.dma_start(
                        et[:],
                        embt_in[:, vc * 512 : (vc + 1) * 512]
                        .rearrange("(o p) c -> p o c", p=P)
                        .bitcast(F32R),
                    )
                    for r in range(4):
                        for tt in range(2):
                            ti = r * 2 + tt
                            pl = ps_mm.tile([P, 512], F32, tag="mm")
                            for k in range(KT):
                                nc.tensor.matmul(
                                    pl[:],
                                    xlg[:, r, k, tt * P : (tt + 1) * P],
                                    et[:, k, :],
                                    start=(k == 0),
                                    stop=(k == KT - 1),
                                )
                            lsb = drains.tile([P, 512], F32, tag="lg")
                            nc.vector.tensor_tensor(
                                lsb[:], pl[:],
                                bias_bc[:, vc * 512 : (vc + 1) * 512],
                                op=mybir.AluOpType.add,
                            )
                            nc.sync.dma_start(
                                logits_out[
                                    ti * P : (ti + 1) * P, vc * 512 : (vc + 1) * 512
                                ],
                                lsb[:],
                            )
                            esc = drains.tile([P, 512], F32, tag="esc")
                            nc.scalar.activation(
                                esc[:], lsb[:], mybir.ActivationFunctionType.Exp,
                                accum_out=sump[:, ti, vc : vc + 1],
                            )
                stile = lmp.tile([P, 8], F32)
                for ti in range(8):
                    nc.vector.tensor_reduce(
                        stile[:, ti : ti + 1], sump[:, ti, :],
                        mybir.AxisListType.X, mybir.AluOpType.add,
                    )
                nc.sync.dma_start(sumexp_out[:].rearrange("a p -> p a"), stile[:])

    nc.finalize()
    _cache["nc"] = nc
    return nc


def _prep_host(inputs):
    f32 = np.float32
    bf16 = ml_dtypes.bfloat16
    idx = np.asarray(inputs["idx"])
    targets = np.asarray(inputs["targets"])
    tok_emb = np.asarray(inputs["tok_emb"], dtype=f32)
    pos_emb = np.asarray(inputs["pos_emb"], dtype=f32)

    qkv_wT = np.ascontiguousarray(
        np.transpose(np.asarray(inputs["qkv_w"], dtype=f32), (0, 2, 1))
    )
    qkv_wT[:, :, :D] *= f32(1.0 / np.sqrt(DH))
    bqkv = np.array(np.asarray(inputs["qkv_b"], dtype=f32))
    bqkv[:, :D] *= f32(1.0 / np.sqrt(DH))
    def featmaj(a, width):
        # [L, M] -> [P, L*width] where column l*width+mo holds a[l, mo*128+p]
        a = np.asarray(a, dtype=f32)
        Lw = a.shape[0]
        out = a.reshape(Lw, width, P).transpose(2, 0, 1).reshape(P, Lw * width)
        return np.ascontiguousarray(out)

    shared = {
        "wqkv": qkv_wT.astype(bf16),
        "wout": np.ascontiguousarray(
            np.transpose(np.asarray(inputs["out_w"], dtype=f32), (0, 2, 1))
        ).astype(bf16),
        "wff1": np.ascontiguousarray(
            np.transpose(np.asarray(inputs["ff1_w"], dtype=f32), (0, 2, 1))
        ).astype(bf16),
        "wff2": np.ascontiguousarray(
            np.transpose(np.asarray(inputs["ff2_w"], dtype=f32), (0, 2, 1))
        ).astype(bf16),
        "bqkv": featmaj(bqkv, 24),
        "bout": featmaj(inputs["out_b"], KT),
        "bff1": featmaj(inputs["ff1_b"], 32),
        "bff2": featmaj(inputs["ff2_b"], KT),
        "ln1w": featmaj(inputs["ln1_w"], KT),
        "ln1b": featmaj(inputs["ln1_b"], KT),
        "ln2w": featmaj(inputs["ln2_w"], KT),
        "ln2b": featmaj(inputs["ln2_b"], KT),
        "lnfw": featmaj(np.asarray(inputs["lnf_w"], dtype=f32).reshape(1, D), KT),
        "lnfb": featmaj(np.asarray(inputs["lnf_b"], dtype=f32).reshape(1, D), KT),
        "ones1": np.ones((P, 1), dtype=f32),
        "identb": np.eye(P, dtype=f32).astype(bf16),
    }

    x0_full = tok_emb[idx] + pos_emb[None, :, :]  # [B, T, D]

    embT = np.ascontiguousarray(tok_emb.T)  # [D, V]
    embT_pad = np.zeros((D, V_OFFS[-1] + VPAD), dtype=f32)
    embT_pad[:, :V] = embT
    lm_b = np.asarray(inputs["lm_head_b"], dtype=f32)
    blm_pad = np.full((V_OFFS[-1] + VPAD,), -1e30, dtype=f32)
    blm_pad[:V] = lm_b

    in_maps = []
    for c in range(8):
        g, s = divmod(c, 4)
        x0 = np.ascontiguousarray(x0_full[g, s * TS : (s + 1) * TS, :].T)
        tk = np.arange(T).reshape(4, 2, P)
        tq = s * TS + np.arange(TS)
        maskt = (tk[:, :, :, None] <= tq[None, None, None, :]).astype(bf16)
        maskt = np.ascontiguousarray(maskt.transpose(2, 0, 1, 3))
        m = dict(shared)
        m["x0"] = x0
        m["maskt"] = np.ascontiguousarray(maskt)
        m["embt"] = np.ascontiguousarray(embT_pad[:, V_OFFS[s] : V_OFFS[s] + VPAD])
        m["blm"] = np.broadcast_to(
            blm_pad[V_OFFS[s] : V_OFFS[s] + VPAD].astype(bf16), (P, VPAD)
        ).copy()
        in_maps.append(m)
    return in_maps, idx, targets


def kernel(**inputs):
    nc = _build_nc()
    in_maps, idx, targets = _prep_host(inputs)
    trace = _cache.get("trace", False)
    res = run_bass_kernel_spmd(nc, in_maps, list(range(8)), trace=trace)
    _cache["last_res"] = res

    logits = np.empty((B, T, V), dtype=np.float32)
    sumexp = np.zeros((B, T), dtype=np.float64)
    for c in range(8):
        g, s = divmod(c, 4)
        r = res.results[c]
        logits[g, :, V_OFFS[s] : V_OFFS[s] + V_LENS[s]] = r["logits"][:, : V_LENS[s]]
        sumexp[g] += r["sumexp"].reshape(T).astype(np.float64)

    logZ = np.log(sumexp)
    tgt = np.take_along_axis(
        logits.astype(np.float64),
        np.asarray(targets)[..., None].astype(np.int64),
        axis=2,
    )[..., 0]
    loss = np.float32(np.mean(logZ - tgt))
    return logits, loss
